# revision 21
# baseline (speedup 1.0000x reference)
"""Multi-head attention (b=8, n=1024, d=768, h=12) on 8 Trainium2 NeuronCores.

Strategy: pure data parallelism over the batch — core i computes batch element
i end-to-end (no collectives). Inside each core the computation is laid out
"feature-major" so no on-chip transposes are ever needed:

  - host passes x^T, w_qkv^T, w_proj^T (transposing on host is free input
    marshalling; the PE contracts over the partition dim so both matmul
    operands need the contraction dim partition-major)
  - q^T/k^T are computed feature-major (qkv^T = w_qkv^T.T @ x^T) so S^T tiles
    come straight out of the PE with keys on partitions and queries on the
    free dim; V is computed token-major (x^T.T @ w_v^T) so it is directly the
    PV lhsT. A ones column appended to each head's V makes row 64 of the PV
    accumulator the softmax denominator.
  - softmax is unnormalized exp (logits are O(5) here, exp cannot overflow);
    scale 1/8 is folded into the ACT exp instruction; normalization happens
    after PV as a per-query reciprocal multiply.
  - v_bias and b_proj fold into one effective bias beff = w_proj @ v_bias +
    b_proj added during the projection PSUM->SBUF copy (sum_j softmax = 1).

Emission order is tuned so the ACT exp stream (the phase-B bottleneck
engine) starts ~10us into the kernel and never waits on phase-A work: pair
0's S/exp tiles are emitted before the V matmuls, PV psum accumulators are
drained to SBUF immediately so the 2-slot PSUM rotation never blocks the
next pair, and q/k tiles are produced two pairs ahead.
"""

import sys

sys.path.insert(0, "/opt/trn_rl_repo")

import os

import numpy as np
import ml_dtypes

import concourse.bass as bass  # noqa: F401  (import keeps bass registered)
import concourse.mybir as mybir
import concourse.tile as tile
from concourse import bacc
from concourse.bass_utils import run_bass_kernel_spmd

N_CORES = 8
B, N, D = 8, 1024, 768
H, HD = 12, 64
SCALE = HD**-0.5
P = 128
KT = D // P  # 6 contraction tiles over d_model
MQ = 2 * D // P  # 12 output tiles over q+k features
TT = N // P  # 8 token tiles
F32 = mybir.dt.float32

# Matmul-operand dtype: "bf16" (bfloat16 operands, fp32 accumulate) or
# "f32r" (fp32 data run through the PE's fast fp32 mode).
DT_MODE = os.environ.get("ATTN_DT", "bf16")


def _np_mm_dtype():
    return ml_dtypes.bfloat16 if DT_MODE == "bf16" else np.float32


def _mm_dt():
    return mybir.dt.bfloat16 if DT_MODE == "bf16" else mybir.dt.float32


def _mm(ap):
    """View an SBUF AP with the dtype actually fed to the tensor engine."""
    if DT_MODE == "f32r":
        return ap.bitcast(mybir.dt.float32r)
    return ap


def build():
    nc = bacc.Bacc("TRN2", target_bir_lowering=False, debug=False)
    dt = _mm_dt()
    Exp = mybir.ActivationFunctionType.Exp

    xT_d = nc.dram_tensor("xT", [D, N], dt, kind="ExternalInput")
    wqkT_d = nc.dram_tensor("wqkT", [D, 2 * D], dt, kind="ExternalInput")
    wvT_d = nc.dram_tensor("wvT", [D, D], dt, kind="ExternalInput")
    wpT_d = nc.dram_tensor("wpT", [D, D], dt, kind="ExternalInput")
    qb_d = nc.dram_tensor("qb", [KT, P, 1], F32, kind="ExternalInput")
    beff_d = nc.dram_tensor("beff", [P, D], F32, kind="ExternalInput")
    out_d = nc.dram_tensor("out", [N, D], F32, kind="ExternalOutput")

    with tile.TileContext(nc) as tc:
        with (
            tc.tile_pool(name="psum", bufs=1, space="PSUM") as psum,
            tc.tile_pool(name="persist", bufs=1) as persist,
            tc.tile_pool(name="work", bufs=1) as work,
        ):
            # ---- input DMAs (ordered by first use) -----------------------------
            x_sb, wqk_sb, wv_sb, qb_sb = [], [], [], []
            for k in range(KT):
                xk = persist.tile([P, N], dt, tag=f"x{k}", name=f"x{k}")
                nc.sync.dma_start(xk[:], xT_d.ap()[k * P : (k + 1) * P, :])
                x_sb.append(xk)
            for k in range(KT):
                wqkk = persist.tile([P, 2 * D], dt, tag=f"wqk{k}", name=f"wqk{k}")
                nc.sync.dma_start(wqkk[:], wqkT_d.ap()[k * P : (k + 1) * P, :])
                wqk_sb.append(wqkk)
            for k in range(KT):
                wvk = persist.tile([P, D], dt, tag=f"wv{k}", name=f"wv{k}")
                nc.sync.dma_start(wvk[:], wvT_d.ap()[k * P : (k + 1) * P, :])
                wv_sb.append(wvk)
            for k in range(KT):
                qbk = persist.tile([P, 1], F32, tag=f"qb{k}", name=f"qb{k}")
                nc.sync.dma_start(qbk[:], qb_d.ap()[k])
                qb_sb.append(qbk)
            wp_sb = []
            for k in range(KT):
                wpk = work.tile([P, D], dt, tag=f"wp{k}", name=f"wp{k}")
                nc.sync.dma_start(wpk[:], wpT_d.ap()[k * P : (k + 1) * P, :])
                wp_sb.append(wpk)
            bb = work.tile([P, D], F32, tag="bb", name="bb")
            nc.sync.dma_start(bb[:], beff_d.ap())

            # ---- emitters ------------------------------------------------------
            v_sb = [None] * TT

            def emit_v(t):
                vt = persist.tile([P, H, HD + 1], dt, tag=f"v{t}", name=f"v{t}")
                nc.vector.memset(vt[:, :, HD], 1.0)
                ps = psum.tile([P, N], F32, tag="mm2", bufs=2, name=f"vps{t}")
                for k in range(KT):
                    lhsT = _mm(x_sb[k])[:, t * P : (t + 1) * P]
                    nc.tensor.matmul(
                        ps[:, 0:512], lhsT, _mm(wv_sb[k])[:, 0:512],
                        start=(k == 0), stop=(k == KT - 1),
                    )
                    nc.tensor.matmul(
                        ps[:, 512:768], lhsT, _mm(wv_sb[k])[:, 512:768],
                        start=(k == 0), stop=(k == KT - 1),
                    )
                nc.vector.tensor_copy(
                    vt[:, :, 0:HD], ps[:, 0:768].rearrange("p (h d) -> p h d", d=HD)
                )
                v_sb[t] = vt

            qk_sb = [None] * MQ

            def emit_qk(m):
                ps = psum.tile([P, N], F32, tag="mm2", bufs=2, name=f"qkps{m}")
                for k in range(KT):
                    lhsT = _mm(wqk_sb[k])[:, m * P : (m + 1) * P]
                    for half in range(2):
                        nc.tensor.matmul(
                            ps[:, half * 512 : (half + 1) * 512],
                            lhsT,
                            _mm(x_sb[k])[:, half * 512 : (half + 1) * 512],
                            start=(k == 0), stop=(k == KT - 1),
                        )
                qkm = persist.tile([P, N], dt, tag=f"qk{m}", name=f"qk{m}")
                if m < KT:
                    # q tile: add q_bias (per-partition scalar in feature-major);
                    # on DVE to keep ACT free for the softmax exps
                    nc.vector.tensor_scalar_add(qkm[:], ps[:], qb_sb[m][:])
                else:
                    nc.vector.tensor_copy(qkm[:], ps[:])
                qk_sb[m] = qkm

            attn_sb = [
                persist.tile([P, N], dt, tag=f"attn{g}", name=f"attn{g}")
                for g in range(KT)
            ]
            pt_tiles = {}  # (g, j, hh) -> exp'd S^T tile, consumed by emit_pv

            def emit_sx(g, j):
                """S^T matmuls + exp for pair g, key tile j (both heads)."""
                q_t, k_t = qk_sb[g], qk_sb[KT + g]
                sps = []
                for hh in range(2):
                    sp = psum.tile(
                        [P, N], F32, tag="sp", bufs=2, name=f"sp{g}_{j}_{hh}"
                    )
                    lhsT = _mm(k_t)[hh * HD : (hh + 1) * HD, j * P : (j + 1) * P]
                    rhs = _mm(q_t)[hh * HD : (hh + 1) * HD, :]
                    # explicit row groups: the two heads' K=64 matmuls
                    # occupy disjoint halves of the PE array and overlap
                    tp = (hh * HD, 0)
                    nc.tensor.matmul(
                        sp[:, 0:512], lhsT, rhs[:, 0:512], tile_position=tp
                    )
                    nc.tensor.matmul(
                        sp[:, 512:1024], lhsT, rhs[:, 512:1024], tile_position=tp
                    )
                    sps.append(sp)
                for hh in range(2):
                    pt = work.tile(
                        [P, N], dt, tag="pt", bufs=16, name=f"pt{g}_{j}_{hh}"
                    )
                    # exp(S^T / 8) straight out of PSUM
                    nc.scalar.activation(pt[:], sps[hh][:], Exp, scale=SCALE)
                    pt_tiles[(g, j, hh)] = pt

            def emit_pv(g):
                """PV accumulation + softmax normalization for head pair g."""
                pp = [
                    psum.tile([HD + 1, N], F32, tag="mm2", bufs=2, name=f"pv{g}_{hh}")
                    for hh in range(2)
                ]
                for j in range(TT):
                    for hh in range(2):
                        pt = pt_tiles.pop((g, j, hh))
                        lhsT = _mm(v_sb[j])[:, 2 * g + hh, :]  # [128, 65] V|1
                        nc.tensor.matmul(
                            pp[hh][:, 0:512], lhsT, _mm(pt)[:, 0:512],
                            start=(j == 0), stop=(j == TT - 1),
                        )
                        nc.tensor.matmul(
                            pp[hh][:, 512:1024], lhsT, _mm(pt)[:, 512:1024],
                            start=(j == 0), stop=(j == TT - 1),
                        )
                # drain PSUM accumulators to SBUF right away so the 2-slot
                # "mm2" rotation unblocks the next pair's q/k matmuls
                ppsb = []
                for hh in range(2):
                    sb = work.tile([HD + 1, N], F32, tag="ppsb", bufs=2, name=f"ppsb{g}{hh}")
                    nc.vector.tensor_copy(sb[:], pp[hh][:])
                    ppsb.append(sb)
                # normalize: row HD of ppsb is the softmax denominator
                for hh in range(2):
                    rrow0 = work.tile([1, N], F32, tag="rr0", bufs=2, name=f"rr0{g}{hh}")
                    rb = work.tile([HD, N], F32, tag="rb", bufs=2, name=f"rb{g}{hh}")
                    # s hops to partition 0 (DMA moves across lanes)...
                    nc.sync.dma_start(rrow0[:], ppsb[hh][HD : HD + 1, :])
                    # ...1/s at base 0 (approx_fast is ~51 ULP, far below the
                    # bf16 noise floor, 5x cheaper than InstReciprocal, and
                    # only correct on base-partition-0 APs)...
                    nc.vector.reciprocal_approx_fast(rrow0[:], rrow0[:])
                    # ...and fan out to all 64 lanes (gpsimd reads partition 0)
                    nc.gpsimd.partition_broadcast(rb[:], rrow0[:])
                    if hh == 0:
                        nc.vector.tensor_mul(
                            attn_sb[g][0:HD, :], ppsb[hh][0:HD, :], rb[:]
                        )
                    else:
                        # PV output lives at partitions 0..64 but this head's
                        # slot in attn_sb is partitions 64..127; DVE lanes
                        # can't shift partitions, so normalize at base 0 and
                        # bounce across partitions with an SBUF->SBUF DMA.
                        stg = work.tile([HD, N], dt, tag="stg", bufs=2, name=f"stg{g}")
                        nc.vector.tensor_mul(stg[:], ppsb[hh][0:HD, :], rb[:])
                        nc.sync.dma_start(attn_sb[g][HD:P, :], stg[:])

            # projection is accumulated per head pair into SBUF so it hides
            # under the next pair's softmax work instead of trailing the
            # kernel; the effective bias seeds the accumulator on pair 0.
            out_acc = [
                persist.tile([P, D], F32, tag=f"oa{t}", name=f"oa{t}")
                for t in range(TT)
            ]

            def emit_proj(g):
                for t in range(TT):
                    ps = psum.tile([P, N], F32, tag="mm2", bufs=2, name=f"ops{g}_{t}")
                    lhsT = _mm(attn_sb[g])[:, t * P : (t + 1) * P]
                    nc.tensor.matmul(ps[:, 0:512], lhsT, _mm(wp_sb[g])[:, 0:512])
                    nc.tensor.matmul(ps[:, 512:768], lhsT, _mm(wp_sb[g])[:, 512:768])
                    src = bb if g == 0 else out_acc[t]
                    nc.vector.tensor_add(out_acc[t][:], ps[:, 0:768], src[:])

            # ---- emission order ------------------------------------------------
            # pair 0's S/exp goes first so the ACT exp stream (the phase-B
            # critical engine) starts ~10us in; all remaining q/k tiles are
            # finished during pair 0's exps so later pairs never add PE work
            # beyond their own S/PV/proj (which fits under the ACT budget).
            emit_qk(0)
            emit_qk(KT)
            for j in range(TT):
                emit_sx(0, j)
            for t in range(TT):
                emit_v(t)
            for g in range(1, KT):
                emit_qk(g)
                emit_qk(KT + g)
            emit_pv(0)
            emit_proj(0)
            for g in range(1, KT):
                for j in range(TT):
                    emit_sx(g, j)
                emit_pv(g)
                emit_proj(g)
            for t in range(TT):
                nc.sync.dma_start(out_d.ap()[t * P : (t + 1) * P, :], out_acc[t][:])

    nc.compile()
    return nc


_NC_CACHE = None


def _get_nc():
    global _NC_CACHE
    if _NC_CACHE is None:
        _NC_CACHE = build()
    return _NC_CACHE


def make_in_maps(x, w_qkv, q_bias, v_bias, w_proj, b_proj):
    mmdt = _np_mm_dtype()
    wqkT = np.ascontiguousarray(w_qkv[: 2 * D].T).astype(mmdt)
    wvT = np.ascontiguousarray(w_qkv[2 * D :].T).astype(mmdt)
    wpT = np.ascontiguousarray(w_proj.T).astype(mmdt)
    qb = np.ascontiguousarray(q_bias.reshape(KT, P, 1)).astype(np.float32)
    beff_row = (
        w_proj.astype(np.float64) @ v_bias.astype(np.float64) + b_proj
    ).astype(np.float32)
    beff = np.ascontiguousarray(np.tile(beff_row, (P, 1)))
    shared = {"wqkT": wqkT, "wvT": wvT, "wpT": wpT, "qb": qb, "beff": beff}
    in_maps = []
    for i in range(N_CORES):
        m = dict(shared)
        m["xT"] = np.ascontiguousarray(x[i].T).astype(mmdt)
        in_maps.append(m)
    return in_maps


def kernel(x, w_qkv, q_bias, v_bias, w_proj, b_proj, _trace=False, _tmpdir=None):
    x = np.asarray(x)
    nc = _get_nc()
    in_maps = make_in_maps(
        np.asarray(x, dtype=np.float32),
        np.asarray(w_qkv, dtype=np.float32),
        np.asarray(q_bias, dtype=np.float32),
        np.asarray(v_bias, dtype=np.float32),
        np.asarray(w_proj, dtype=np.float32),
        np.asarray(b_proj, dtype=np.float32),
    )
    res = run_bass_kernel_spmd(
        nc, in_maps, core_ids=list(range(N_CORES)), trace=_trace, tmpdir=_tmpdir
    )
    out = np.stack([res.results[i]["out"] for i in range(N_CORES)], axis=0)
    if _trace:
        return out.astype(np.float32), res
    return out.astype(np.float32)


# revision 22
# speedup vs baseline: 1.2960x; 1.2960x over previous
"""Multi-head attention (b=8, n=1024, d=768, h=12) on 8 Trainium2 NeuronCores.

Strategy: pure data parallelism over the batch — core i computes batch element
i end-to-end (no collectives). Inside each core the computation is laid out
"feature-major" so no on-chip transposes are ever needed:

  - host passes x^T, w_qkv^T, w_proj^T (transposing on host is free input
    marshalling; the PE contracts over the partition dim so both matmul
    operands need the contraction dim partition-major)
  - q^T/k^T are computed feature-major (qkv^T = w_qkv^T.T @ x^T) so S^T tiles
    come straight out of the PE with keys on partitions and queries on the
    free dim; V is computed token-major (x^T.T @ w_v^T) so it is directly the
    PV lhsT. A ones column appended to each head's V makes row 64 of the PV
    accumulator the softmax denominator.
  - softmax is unnormalized exp (logits are O(5) here, exp cannot overflow);
    scale 1/8 is folded into the ACT exp instruction; normalization happens
    after PV as a per-query reciprocal multiply.
  - v_bias and b_proj fold into one effective bias beff = w_proj @ v_bias +
    b_proj added during the projection PSUM->SBUF copy (sum_j softmax = 1).

Emission order is tuned so the ACT exp stream (the phase-B bottleneck
engine) starts ~10us into the kernel and never waits on phase-A work: pair
0's S/exp tiles are emitted before the V matmuls, PV psum accumulators are
drained to SBUF immediately so the 2-slot PSUM rotation never blocks the
next pair, and q/k tiles are produced two pairs ahead.
"""

import sys

sys.path.insert(0, "/opt/trn_rl_repo")

import os

import numpy as np
import ml_dtypes

import concourse.bass as bass  # noqa: F401  (import keeps bass registered)
import concourse.mybir as mybir
import concourse.tile as tile
from concourse import bacc
from concourse.bass_utils import run_bass_kernel_spmd

N_CORES = 8
B, N, D = 8, 1024, 768
H, HD = 12, 64
SCALE = HD**-0.5
P = 128
KT = D // P  # 6 contraction tiles over d_model
MQ = 2 * D // P  # 12 output tiles over q+k features
TT = N // P  # 8 token tiles
F32 = mybir.dt.float32

# Matmul-operand dtype: "bf16" (bfloat16 operands, fp32 accumulate) or
# "f32r" (fp32 data run through the PE's fast fp32 mode).
DT_MODE = os.environ.get("ATTN_DT", "bf16")


def _np_mm_dtype():
    return ml_dtypes.bfloat16 if DT_MODE == "bf16" else np.float32


def _mm_dt():
    return mybir.dt.bfloat16 if DT_MODE == "bf16" else mybir.dt.float32


def _mm(ap):
    """View an SBUF AP with the dtype actually fed to the tensor engine."""
    if DT_MODE == "f32r":
        return ap.bitcast(mybir.dt.float32r)
    return ap


def build():
    nc = bacc.Bacc("TRN2", target_bir_lowering=False, debug=False)
    dt = _mm_dt()
    Exp = mybir.ActivationFunctionType.Exp

    xT_d = nc.dram_tensor("xT", [D, N], dt, kind="ExternalInput")
    wqkT_d = nc.dram_tensor("wqkT", [D, 2 * D], dt, kind="ExternalInput")
    wvT_d = nc.dram_tensor("wvT", [D, D], dt, kind="ExternalInput")
    wpT_d = nc.dram_tensor("wpT", [D, D], dt, kind="ExternalInput")
    qb_d = nc.dram_tensor("qb", [KT, P, 1], F32, kind="ExternalInput")
    beff_d = nc.dram_tensor("beff", [P, D], F32, kind="ExternalInput")
    out_d = nc.dram_tensor("out", [N, D], F32, kind="ExternalOutput")

    with tile.TileContext(nc) as tc:
        with (
            tc.tile_pool(name="psum", bufs=1, space="PSUM") as psum,
            tc.tile_pool(name="persist", bufs=1) as persist,
            tc.tile_pool(name="work", bufs=1) as work,
        ):
            # ---- input DMAs (ordered by first use) -----------------------------
            x_sb, wqk_sb, wv_sb, qb_sb = [], [], [], []
            for k in range(KT):
                xk = persist.tile([P, N], dt, tag=f"x{k}", name=f"x{k}")
                nc.sync.dma_start(xk[:], xT_d.ap()[k * P : (k + 1) * P, :])
                x_sb.append(xk)
            for k in range(KT):
                wqkk = persist.tile([P, 2 * D], dt, tag=f"wqk{k}", name=f"wqk{k}")
                nc.sync.dma_start(wqkk[:], wqkT_d.ap()[k * P : (k + 1) * P, :])
                wqk_sb.append(wqkk)
            for k in range(KT):
                wvk = persist.tile([P, D], dt, tag=f"wv{k}", name=f"wv{k}")
                nc.sync.dma_start(wvk[:], wvT_d.ap()[k * P : (k + 1) * P, :])
                wv_sb.append(wvk)
            for k in range(KT):
                qbk = persist.tile([P, 1], F32, tag=f"qb{k}", name=f"qb{k}")
                nc.sync.dma_start(qbk[:], qb_d.ap()[k])
                qb_sb.append(qbk)
            wp_sb = []
            for k in range(KT):
                wpk = work.tile([P, D], dt, tag=f"wp{k}", name=f"wp{k}")
                nc.sync.dma_start(wpk[:], wpT_d.ap()[k * P : (k + 1) * P, :])
                wp_sb.append(wpk)
            bb = work.tile([P, D], F32, tag="bb", name="bb")
            nc.sync.dma_start(bb[:], beff_d.ap())

            # ---- emitters ------------------------------------------------------
            v_sb = [None] * TT

            def emit_v(t):
                vt = persist.tile([P, H, HD + 1], dt, tag=f"v{t}", name=f"v{t}")
                nc.vector.memset(vt[:, :, HD], 1.0)
                ps = psum.tile([P, N], F32, tag="mm2", bufs=2, name=f"vps{t}")
                for k in range(KT):
                    lhsT = _mm(x_sb[k])[:, t * P : (t + 1) * P]
                    nc.tensor.matmul(
                        ps[:, 0:512], lhsT, _mm(wv_sb[k])[:, 0:512],
                        start=(k == 0), stop=(k == KT - 1),
                    )
                    nc.tensor.matmul(
                        ps[:, 512:768], lhsT, _mm(wv_sb[k])[:, 512:768],
                        start=(k == 0), stop=(k == KT - 1),
                    )
                nc.vector.tensor_copy(
                    vt[:, :, 0:HD], ps[:, 0:768].rearrange("p (h d) -> p h d", d=HD)
                )
                v_sb[t] = vt

            qk_sb = [None] * MQ

            def emit_qk(m):
                ps = psum.tile([P, N], F32, tag="mm2", bufs=2, name=f"qkps{m}")
                for k in range(KT):
                    lhsT = _mm(wqk_sb[k])[:, m * P : (m + 1) * P]
                    for half in range(2):
                        nc.tensor.matmul(
                            ps[:, half * 512 : (half + 1) * 512],
                            lhsT,
                            _mm(x_sb[k])[:, half * 512 : (half + 1) * 512],
                            start=(k == 0), stop=(k == KT - 1),
                        )
                qkm = persist.tile([P, N], dt, tag=f"qk{m}", name=f"qk{m}")
                if m < KT:
                    # q tile: add q_bias (per-partition scalar in feature-major);
                    # on DVE to keep ACT free for the softmax exps
                    nc.vector.tensor_scalar_add(qkm[:], ps[:], qb_sb[m][:])
                else:
                    nc.vector.tensor_copy(qkm[:], ps[:])
                qk_sb[m] = qkm

            attn_sb = [
                persist.tile([P, N], dt, tag=f"attn{g}", name=f"attn{g}")
                for g in range(KT)
            ]
            pt_tiles = {}  # (g, j, hh) -> exp'd S^T tile, consumed by emit_pv

            def emit_sx(g, j):
                """S^T matmuls + exp for pair g, key tile j (both heads)."""
                q_t, k_t = qk_sb[g], qk_sb[KT + g]
                sps = []
                for hh in range(2):
                    sp = psum.tile(
                        [P, N], F32, tag="sp", bufs=2, name=f"sp{g}_{j}_{hh}"
                    )
                    lhsT = _mm(k_t)[hh * HD : (hh + 1) * HD, j * P : (j + 1) * P]
                    rhs = _mm(q_t)[hh * HD : (hh + 1) * HD, :]
                    # explicit row groups: the two heads' K=64 matmuls
                    # occupy disjoint halves of the PE array and overlap
                    tp = (hh * HD, 0)
                    nc.tensor.matmul(
                        sp[:, 0:512], lhsT, rhs[:, 0:512], tile_position=tp
                    )
                    nc.tensor.matmul(
                        sp[:, 512:1024], lhsT, rhs[:, 512:1024], tile_position=tp
                    )
                    sps.append(sp)
                for hh in range(2):
                    pt = work.tile(
                        [P, N], dt, tag="pt", bufs=16, name=f"pt{g}_{j}_{hh}"
                    )
                    # exp(S^T / 8) straight out of PSUM
                    nc.scalar.activation(pt[:], sps[hh][:], Exp, scale=SCALE)
                    pt_tiles[(g, j, hh)] = pt

            def emit_pv(g):
                """PV accumulation + softmax normalization for head pair g."""
                pp = [
                    psum.tile([HD + 1, N], F32, tag="mm2", bufs=2, name=f"pv{g}_{hh}")
                    for hh in range(2)
                ]
                for j in range(TT):
                    for hh in range(2):
                        pt = pt_tiles.pop((g, j, hh))
                        lhsT = _mm(v_sb[j])[:, 2 * g + hh, :]  # [128, 65] V|1
                        nc.tensor.matmul(
                            pp[hh][:, 0:512], lhsT, _mm(pt)[:, 0:512],
                            start=(j == 0), stop=(j == TT - 1),
                        )
                        nc.tensor.matmul(
                            pp[hh][:, 512:1024], lhsT, _mm(pt)[:, 512:1024],
                            start=(j == 0), stop=(j == TT - 1),
                        )
                # drain PSUM accumulators to SBUF right away so the 2-slot
                # "mm2" rotation unblocks the next pair's q/k matmuls
                ppsb = []
                for hh in range(2):
                    sb = work.tile([HD + 1, N], F32, tag="ppsb", bufs=2, name=f"ppsb{g}{hh}")
                    nc.vector.tensor_copy(sb[:], pp[hh][:])
                    ppsb.append(sb)
                # normalize: row HD of ppsb is the softmax denominator
                for hh in range(2):
                    rrow0 = work.tile([1, N], F32, tag="rr0", bufs=2, name=f"rr0{g}{hh}")
                    rb = work.tile([HD, N], F32, tag="rb", bufs=2, name=f"rb{g}{hh}")
                    # s hops to partition 0 (DMA moves across lanes)...
                    nc.sync.dma_start(rrow0[:], ppsb[hh][HD : HD + 1, :])
                    # ...1/s at base 0 (approx_fast is ~51 ULP, far below the
                    # bf16 noise floor, 5x cheaper than InstReciprocal, and
                    # only correct on base-partition-0 APs)...
                    nc.vector.reciprocal_approx_fast(rrow0[:], rrow0[:])
                    # ...and fan out to all 64 lanes (gpsimd reads partition 0)
                    nc.gpsimd.partition_broadcast(rb[:], rrow0[:])
                    if hh == 0:
                        nc.vector.tensor_mul(
                            attn_sb[g][0:HD, :], ppsb[hh][0:HD, :], rb[:]
                        )
                    else:
                        # PV output lives at partitions 0..64 but this head's
                        # slot in attn_sb is partitions 64..127; DVE lanes
                        # can't shift partitions, so normalize at base 0 and
                        # bounce across partitions with an SBUF->SBUF DMA.
                        stg = work.tile([HD, N], dt, tag="stg", bufs=2, name=f"stg{g}")
                        nc.vector.tensor_mul(stg[:], ppsb[hh][0:HD, :], rb[:])
                        nc.sync.dma_start(attn_sb[g][HD:P, :], stg[:])

            # ---- emission order ------------------------------------------------
            # Tile's per-engine instruction order is (near-)emission order and
            # a stalled instruction head-of-line blocks its engine, so filler
            # work (V) is emitted BEFORE each exp-paced S step.
            emit_qk(0)
            emit_qk(KT)
            for j in range(TT):
                emit_v(j)
                emit_sx(0, j)
            for g in range(1, KT):
                emit_qk(g)
                emit_qk(KT + g)
            emit_pv(0)
            for g in range(1, KT):
                for j in range(TT):
                    emit_sx(g, j)
                emit_pv(g)

            # ---- phase C: out = attn @ w_proj^T + beff -------------------------
            for t in range(TT):
                ps = psum.tile([P, N], F32, tag="mm2", bufs=2, name=f"ops{t}")
                for k in range(KT):
                    lhsT = _mm(attn_sb[k])[:, t * P : (t + 1) * P]
                    nc.tensor.matmul(
                        ps[:, 0:512], lhsT, _mm(wp_sb[k])[:, 0:512],
                        start=(k == 0), stop=(k == KT - 1),
                    )
                    nc.tensor.matmul(
                        ps[:, 512:768], lhsT, _mm(wp_sb[k])[:, 512:768],
                        start=(k == 0), stop=(k == KT - 1),
                    )
                ot = work.tile([P, D], F32, tag="ot", bufs=3, name=f"ot{t}")
                nc.vector.tensor_add(ot[:], ps[:, 0:768], bb[:])
                nc.sync.dma_start(out_d.ap()[t * P : (t + 1) * P, :], ot[:])

    nc.compile()
    return nc


_NC_CACHE = None


def _get_nc():
    global _NC_CACHE
    if _NC_CACHE is None:
        _NC_CACHE = build()
    return _NC_CACHE


def make_in_maps(x, w_qkv, q_bias, v_bias, w_proj, b_proj):
    mmdt = _np_mm_dtype()
    wqkT = np.ascontiguousarray(w_qkv[: 2 * D].T).astype(mmdt)
    wvT = np.ascontiguousarray(w_qkv[2 * D :].T).astype(mmdt)
    wpT = np.ascontiguousarray(w_proj.T).astype(mmdt)
    qb = np.ascontiguousarray(q_bias.reshape(KT, P, 1)).astype(np.float32)
    beff_row = (
        w_proj.astype(np.float64) @ v_bias.astype(np.float64) + b_proj
    ).astype(np.float32)
    beff = np.ascontiguousarray(np.tile(beff_row, (P, 1)))
    shared = {"wqkT": wqkT, "wvT": wvT, "wpT": wpT, "qb": qb, "beff": beff}
    in_maps = []
    for i in range(N_CORES):
        m = dict(shared)
        m["xT"] = np.ascontiguousarray(x[i].T).astype(mmdt)
        in_maps.append(m)
    return in_maps


def kernel(x, w_qkv, q_bias, v_bias, w_proj, b_proj, _trace=False, _tmpdir=None):
    x = np.asarray(x)
    nc = _get_nc()
    in_maps = make_in_maps(
        np.asarray(x, dtype=np.float32),
        np.asarray(w_qkv, dtype=np.float32),
        np.asarray(q_bias, dtype=np.float32),
        np.asarray(v_bias, dtype=np.float32),
        np.asarray(w_proj, dtype=np.float32),
        np.asarray(b_proj, dtype=np.float32),
    )
    res = run_bass_kernel_spmd(
        nc, in_maps, core_ids=list(range(N_CORES)), trace=_trace, tmpdir=_tmpdir
    )
    out = np.stack([res.results[i]["out"] for i in range(N_CORES)], axis=0)
    if _trace:
        return out.astype(np.float32), res
    return out.astype(np.float32)
